# revision 12
# baseline (speedup 1.0000x reference)
"""GAT (graph attention) forward on 8 TRN2 NeuronCores, Bass/Tile.

Sharding: target nodes (rows of the output) split into 8 blocks of 512.
Each core redundantly computes the projected features h for ALL nodes
(cheap: one K=128 matmul chain) and then its own 512-row slice of the
attention + aggregation + skip + ELU.  No collectives.

Score layout trick: scores are built in [m, n] layout (source node m on
partitions, local target n on the free dim) so the unnormalized attention
tile is already transposed for the PE aggregation matmul
    out_ext^T[f, n] = sum_m h_ext[m, f] * exp_scores[m, n]
where h_ext has a ones-column appended (f = 64) so the softmax
denominator falls out of the same matmul.  Softmax skips the max-shift:
logits are O(10), masked entries hold -1e9 and underflow exp -> exactly 0.

The mask is transposed on the host per core so the device only ever does
contiguous row DMA.

All DMA goes through SWDGE (gpsimd.dma_start) and PE-read constants are
packed into one DRAM parameter: the S3_LW (weight-load) instruction can
carry only one semaphore wait, so every matmul must depend on at most
one producer semaphore.  Cheap PE warmup ops absorb first-touch waits.
"""

import numpy as np
from contextlib import ExitStack

import concourse.bass as bass
import concourse.mybir as mybir
from concourse.tile import TileContext
from concourse.masks import make_identity
from concourse.bass_utils import run_bass_kernel_spmd

F32 = mybir.dt.float32
AF = mybir.ActivationFunctionType
OP = mybir.AluOpType

N, FIN, H, FOUT = 4096, 128, 4, 64
G = H * FOUT
NCORES = 8
NLOC = N // NCORES          # local target rows per core
NCH = N // 128              # source (m) chunks
LCH = NLOC // 128           # local output row chunks


def build_program(n=N, h_heads=H, fout=FOUT, nloc=NLOC):
    g = h_heads * fout
    nch = n // 128
    lch = nloc // 128
    he = fout + 1            # h_ext columns (ones col at index fout)
    # cpack: xT | wproj | wsc | wskip | xTloc | biasb  (one DMA, one sem)
    cw = n + g + 2 * h_heads + g + nloc + g

    nc = bass.Bass()
    d_cpack = nc.declare_dram_parameter("cpack", [128, cw], F32, isOutput=False)
    d_maskT = nc.declare_dram_parameter("maskT", [n, nloc], F32, isOutput=False)
    d_out = nc.declare_dram_parameter("out", [nloc, g], F32, isOutput=True)

    with TileContext(nc) as tc, ExitStack() as ctx:
        cp = ctx.enter_context(tc.tile_pool(name="const", bufs=1))
        sb_cpack = cp.tile([128, cw], F32, tag="cpack")
        o = 0
        sb_xT = sb_cpack[:, o:o + n]; o += n
        sb_wproj = sb_cpack[:, o:o + g]; o += g
        sb_wsc = sb_cpack[:, o:o + 2 * h_heads]; o += 2 * h_heads
        sb_wskip = sb_cpack[:, o:o + g]; o += g
        sb_xTloc = sb_cpack[:, o:o + nloc]; o += nloc
        sb_biasb = sb_cpack[:, o:o + g]; o += g
        sb_ones = cp.tile([128, 128], F32, tag="ones")
        sb_id = cp.tile([128, 128], F32, tag="ident")
        sb_mask = cp.tile([128, nch * nloc], F32, tag="mask")
        sb_h = cp.tile([128, nch * h_heads * he], F32, tag="hext")
        sb_stgt = cp.tile([128, nch * h_heads], F32, tag="stgt")
        sb_ssrc = cp.tile([128, h_heads * nloc], F32, tag="ssrc")

        nc.gpsimd.dma_start(out=sb_cpack[:], in_=d_cpack[:])
        nc.vector.memset(sb_ones[:], 1.0)
        make_identity(nc, sb_id[:])
        for j in range(nch):
            nc.gpsimd.dma_start(out=sb_mask[:, j * nloc:(j + 1) * nloc],
                                in_=d_maskT[j * 128:(j + 1) * 128, :])

        # h_ext view: [128, nch*h, he]; chunk (j, head) at index j*h + head
        hv = sb_h[:].rearrange("p (c w) -> p c w", w=he)
        nc.vector.memset(hv[:, :, fout:fout + 1], 1.0)

        # ---- phase 1: h = x @ proj (all heads at once) + s_tgt -------------
        with tc.tile_pool(name="ps1", bufs=2, space="PSUM") as ps1:
            for j in range(nch):
                ph = ps1.tile([128, g + h_heads], F32, tag="ph")
                lhs = sb_xT[:, j * 128:(j + 1) * 128]
                nc.tensor.matmul(ph[:, 0:g], lhs, sb_wproj, start=True, stop=True)
                nc.tensor.matmul(ph[:, g:g + h_heads], lhs,
                                 sb_wsc[:, h_heads:2 * h_heads], start=True, stop=True)
                src_h = ph[:, 0:g].rearrange("p (hh f) -> p hh f", f=fout)
                nc.vector.tensor_copy(hv[:, j * h_heads:(j + 1) * h_heads, 0:fout], src_h)
                nc.vector.tensor_copy(sb_stgt[:, j * h_heads:(j + 1) * h_heads],
                                      ph[:, g:g + h_heads])

            # PE warmups: absorb first-touch semaphore waits so that the
            # hot-loop matmuls each carry a single wait (S3_LW limit).
            pscr = ps1.tile([128, 32], F32, tag="pscr")
            nc.tensor.matmul(pscr[0:he, 0:1], hv[:, 0, :], sb_ones[:, 0:1],
                             start=True, stop=True)
            pscr2 = ps1.tile([128, 32], F32, tag="pscr2")
            nc.tensor.transpose(pscr2[0:32, 0:32], sb_id[0:32, 0:32],
                                sb_id[0:32, 0:32])

            # ---- phase 2: s_src broadcast tiles [128, nloc] per head -------
            for hh in range(h_heads):
                tmp = cp.tile([128, nloc], F32, tag="bctmp")
                nc.vector.tensor_scalar(tmp[:], sb_xTloc[:], sb_wsc[:, hh:hh + 1],
                                        None, OP.mult)
                pb = ps1.tile([128, nloc], F32, tag="pb")
                nc.tensor.matmul(pb[:], sb_ones[:], tmp[:], start=True, stop=True)
                nc.scalar.copy(sb_ssrc[:, hh * nloc:(hh + 1) * nloc], pb[:])

        # ---- phase 3: attention main loop ---------------------------------
        po = []
        pso = ctx.enter_context(tc.tile_pool(name="pso", bufs=1, space="PSUM"))
        for hh in range(h_heads):
            po.append(pso.tile([128, nloc], F32, tag=f"po{hh}", name=f"po{hh}"))

        with tc.tile_pool(name="work", bufs=4) as wp:
            for hh in range(h_heads):
                ssrc = sb_ssrc[:, hh * nloc:(hh + 1) * nloc]
                for j in range(nch):
                    zt = wp.tile([128, nloc], F32, tag="zt")
                    # z = s_src[n] + s_tgt[m]  (ACT, bias = per-partition scalar)
                    nc.scalar.activation(zt[:], ssrc, AF.Identity,
                                         bias=sb_stgt[:, j * h_heads + hh:
                                                      j * h_heads + hh + 1])
                    # leaky_relu(z, 0.2) = max(z, 0.2 z)
                    lt = wp.tile([128, nloc], F32, tag="lt")
                    nc.vector.tensor_scalar(lt[:], zt[:], 0.2, None, OP.mult)
                    nc.vector.tensor_tensor(zt[:], zt[:], lt[:], OP.max)
                    # + mask (0 / -1e9), then exp
                    nc.vector.tensor_tensor(zt[:], zt[:],
                                            sb_mask[:, j * nloc:(j + 1) * nloc], OP.add)
                    et = wp.tile([128, nloc], F32, tag="et")
                    nc.scalar.activation(et[:], zt[:], AF.Exp)
                    nc.tensor.matmul(po[hh][0:he, :],
                                     hv[:, j * h_heads + hh, :], et[:],
                                     start=(j == 0), stop=(j == nch - 1))

            # ---- phase 4/5: normalize, transpose, skip, bias, ELU ---------
            pon = []
            for hh in range(h_heads):
                pos = cp.tile([128, nloc], F32, tag=f"pos{hh}", name=f"pos{hh}")
                nc.scalar.copy(pos[0:he, :], po[hh][0:he, :])
                pon.append(pos)

        with tc.tile_pool(name="fin", bufs=2) as fp, \
             tc.tile_pool(name="psf", bufs=2, space="PSUM") as psf:
            for li in range(lch):
                af = fp.tile([128, g], F32, tag="af")
                for hh in range(h_heads):
                    pt = psf.tile([128, he], F32, tag="pt")
                    nc.tensor.transpose(pt[0:128, 0:he],
                                        pon[hh][0:he, li * 128:(li + 1) * 128],
                                        sb_id[0:he, 0:he])
                    rcp = fp.tile([128, 1], F32, tag="rcp")
                    nc.vector.reciprocal(rcp[:], pt[:, fout:fout + 1])
                    nc.vector.tensor_scalar(af[:, hh * fout:(hh + 1) * fout],
                                            pt[:, 0:fout], rcp[:], None, OP.mult)
                # skip connection: x_loc_chunk @ skip_w.T  (+ bias)
                pskip = psf.tile([128, g], F32, tag="pskip")
                nc.tensor.matmul(pskip[:], sb_xTloc[:, li * 128:(li + 1) * 128],
                                 sb_wskip, start=True, stop=True)
                nc.vector.tensor_tensor(af[:], af[:], pskip[:], OP.add)
                nc.vector.tensor_tensor(af[:], af[:], sb_biasb[:], OP.add)
                # ELU(z) = max(z,0) + exp(min(z,0)) - 1
                mn = fp.tile([128, g], F32, tag="mn")
                nc.vector.tensor_scalar(mn[:], af[:], 0.0, None, OP.min)
                ex = fp.tile([128, g], F32, tag="ex")
                nc.scalar.activation(ex[:], mn[:], AF.Exp)
                nc.vector.tensor_scalar(af[:], af[:], 0.0, None, OP.max)
                nc.vector.tensor_tensor(af[:], af[:], ex[:], OP.add)
                nc.vector.tensor_scalar(af[:], af[:], -1.0, None, OP.add)
                nc.gpsimd.dma_start(out=d_out[li * 128:(li + 1) * 128, :], in_=af[:])

    _split_multi_waits(nc)
    return nc


def _split_multi_waits(nc):
    """walrus on this toolchain allows only one semaphore-wait command on
    most compute-engine instructions (S3_LW / S3D3_* structs).  Tile's
    scheduler freely emits 2+.  Move all but one wait onto an injected
    same-engine NoOp right before the offending instruction."""
    skip = (mybir.InstEventSemaphore,)
    k = 0
    for f in nc.m.functions:
        for blk in f.blocks:
            new = []
            for ins in blk.instructions:
                si = getattr(ins, "sync_info", None)
                w = list(si.on_wait) if si is not None and si.on_wait else []
                if len(w) > 1 and not isinstance(ins, skip):
                    for wx in w[:-1]:
                        nop = mybir.InstNoOp(name=f"waitsplit-{k}", ins=[], outs=[])
                        nop.engine = ins.engine
                        nop.sync_info = mybir.SyncInfo(on_wait=[wx], on_update=[])
                        new.append(nop)
                        k += 1
                    ins.sync_info = mybir.SyncInfo(on_wait=w[-1:],
                                                   on_update=list(si.on_update))
                new.append(ins)
            blk.instructions[:] = new


_PROG = None


def _get_prog():
    global _PROG
    if _PROG is None:
        _PROG = build_program()
    return _PROG


def make_in_maps(x, mask, proj_param, score_src, score_tgt, skip_w, bias):
    x = np.asarray(x, np.float32)
    mask = np.asarray(mask, np.float32)
    proj = np.asarray(proj_param, np.float32)
    a_src = np.asarray(score_src, np.float32)[:, :, 0]       # [H, FOUT]
    a_tgt = np.asarray(score_tgt, np.float32)[:, :, 0]
    skip = np.asarray(skip_w, np.float32)
    b = np.asarray(bias, np.float32)

    xT = np.ascontiguousarray(x.T)                           # [128, N]
    wproj = np.ascontiguousarray(proj.transpose(1, 0, 2).reshape(FIN, G))
    w_src = np.einsum('hif,hf->ih', proj, a_src)             # [FIN, H]
    w_tgt = np.einsum('hif,hf->ih', proj, a_tgt)
    wsc = np.concatenate([w_src, w_tgt], 1).astype(np.float32)
    wskip = np.ascontiguousarray(skip.T)                     # [128, G]
    biasb = np.broadcast_to(b[None, :], (128, G)).astype(np.float32)

    in_maps = []
    for c in range(NCORES):
        r0 = c * NLOC
        cpack = np.ascontiguousarray(np.concatenate(
            [xT, wproj, wsc, wskip, xT[:, r0:r0 + NLOC], biasb], axis=1),
            np.float32)
        in_maps.append({
            "cpack": cpack,
            "maskT": np.ascontiguousarray(mask[r0:r0 + NLOC, :].T),
        })
    return in_maps


def run(in_maps, trace=False, **kw):
    res = run_bass_kernel_spmd(_get_prog(), in_maps, list(range(NCORES)),
                               trace=trace, **kw)
    out = np.concatenate([res.results[c]["out"] for c in range(NCORES)], axis=0)
    return out, res


def kernel(x, mask, proj_param, score_src, score_tgt, skip_w, bias):
    in_maps = make_in_maps(x, mask, proj_param, score_src, score_tgt, skip_w, bias)
    out, _ = run(in_maps)
    return out.astype(np.float32)
